# revision 11
# baseline (speedup 1.0000x reference)
"""Trainium2 Bass kernel for nn_Attn: out = softmax_s(v . (W @ q_s + b)).

Algebraic identity:
    energies[s] = v . (W @ q[s] + b) = q[s] . (W^T v) + (v . b)
The (v . b) term is constant and softmax is shift-invariant, so it drops out.
The kernel computes u = W^T v (tiny), energies = question @ u (a matvec), and
a sharded softmax.

Why NOT the PE array for the matvec: TensorE streams rhs at 1 fp32
column/SBUF-cycle and fp32 matmuls are 2-pass (LOW/HIGH), so pushing all of a
core's q through it costs ~75+ us/core — well above the HBM floor. The DVE
does a fused multiply + free-axis-reduce (scalar_tensor_tensor with
accum_out) at 1 fp32 elem/lane/cycle: a [128,1024] tile in ~1.2 us, 32 tiles
in ~39 us — it hides under the DMA stream.

Why NO collectives: on this runner the 8 NEFFs enter ~60 us apart (the entry
barrier in every traced run spans to ~65 us), so ANY cross-core exchange
stalls the early cores for the skew. Instead every core reads the full W
(+4 MB on the stream, ~12 us) and computes u itself; cores are fully
independent, so each core's exec time is just its own work.

Distribution over 8 NeuronCores — seq (token) sharding, question in its
NATIVE [tokens, H] layout (no host transpose of the big tensor):
  - core r owns tokens [r*4096, (r+1)*4096); partition p holds tokens
    [32p, 32p+32), so every q DMA is 128 partitions x contiguous bytes.
  - q chunks are split across BOTH HWDGE queues (sync/SP and scalar/ACT) to
    get above the ~284 GB/s single-queue ceiling.
  - u: W is host-rearranged to [128 o-part, oc, j] so its DMA is
    128 x 32 KB contiguous; the otherwise-idle PE accumulates
    u = sum_oc W_oc^T v_oc into two [1,512] PSUM banks, pipelined with the
    4 W-piece DMAs; a K=1 ones-matmul broadcasts u to all 128 partitions.
  - energies: 32 DVE scalar_tensor_tensor ops, accum_out -> e_loc[:, t].
  - softmax: per-PARTITION stats only (negmax via DVE reduce, exp + rowsum
    via one ACT activation), packed with the 32 unnormalized exp columns
    into one [128, 34] output DMA.
  - host merge (standard sharded-softmax combine, O(S) data movement):
    M = max m_rp, Sg = sum s_rp*exp(m_rp-M), out = p_un * exp(m_rp-M)/Sg.
"""

import numpy as np

S = 32768
H = 1024
NCORES = 8
TPC = S // NCORES  # 4096 tokens per core
TPT = 32  # tokens (sub-tiles) per partition
NCHUNK = 16  # 1 MB q DMAs per core
SPC = TPT // NCHUNK  # sub-tiles per chunk (2)
OC = H // 128  # 8 o-chunks for the u matmuls
NWPC = 4  # W DMA pieces (1 MB each)

_cached = {}


def _build():
    from contextlib import ExitStack

    import concourse.bass as bass
    import concourse.mybir as mybir
    import concourse.tile as tile
    from concourse import bacc

    f32 = mybir.dt.float32
    AX = mybir.AxisListType
    OP = mybir.AluOpType
    ds = bass.ds

    nc = bacc.Bacc(
        "TRN2", target_bir_lowering=False, debug=False, num_devices=NCORES
    )

    q = nc.dram_tensor("q", [TPC, H], f32, kind="ExternalInput")
    wcat = nc.dram_tensor("wcat", [128, OC * H], f32, kind="ExternalInput")
    vb = nc.dram_tensor("vb", [128, OC * 128], f32, kind="ExternalInput")
    outp = nc.dram_tensor("outp", [128, TPT + 2], f32, kind="ExternalOutput")

    with tile.TileContext(nc) as tc, ExitStack() as ctx:
        const = ctx.enter_context(tc.tile_pool(name="const", bufs=1))
        qpool = ctx.enter_context(tc.tile_pool(name="qpool", bufs=NCHUNK))
        work = ctx.enter_context(tc.tile_pool(name="work", bufs=1))
        scr = ctx.enter_context(tc.tile_pool(name="scr", bufs=2))
        psum_b = ctx.enter_context(tc.tile_pool(name="psum_b", bufs=2, space="PSUM"))

        # --- sync queue: W pieces first (u path), then some q chunks.
        # vb[o, oc*128 + p] = v[128oc + o] (v replicated across 128 columns):
        # a rank-1 lhsT makes each u-matmul write u broadcast to ALL 128
        # output partitions, so PSUM accumulates u_rep directly — no separate
        # broadcast stage.
        vb_sb = const.tile([128, OC * 128], f32)
        nc.sync.dma_start(vb_sb[:], vb[:])
        w_sb = const.tile([128, OC * H], f32)
        WPW = OC * H // NWPC  # elems per W piece
        for w in range(NWPC):
            nc.sync.dma_start(
                w_sb[:, ds(w * WPW, WPW)], wcat[:, ds(w * WPW, WPW)]
            )

        # --- q stream split across three DMA queues: sync + scalar (HWDGE)
        # and gpsimd (SWDGE)
        q_view = q[:].rearrange("(p t) h -> p (t h)", p=128)
        CW = SPC * H
        q_sb = []
        for k in range(NCHUNK):
            t_ = qpool.tile([128, CW], f32, tag="q")
            if k < 12:  # early chunks on the two queues not busy with W
                eng = nc.scalar if k % 2 == 0 else nc.gpsimd
            else:  # late chunks behind W on the sync queue
                eng = nc.sync
            eng.dma_start(t_[:], q_view[:, ds(k * CW, CW)])
            q_sb.append(t_)

        # --- u_rep = (v-replicated)^T @ W on the PE, pipelined with W
        # arrival; two PSUM banks accumulate the two 512-wide halves
        pb0 = psum_b.tile([128, 512], f32, tag="pb0")
        pb1 = psum_b.tile([128, 512], f32, tag="pb1")
        pb = [pb0, pb1]
        for c in range(OC):
            for half in range(2):
                nc.tensor.matmul(
                    pb[half][:],
                    lhsT=vb_sb[:, ds(c * 128, 128)],
                    rhs=w_sb[:, ds(c * H + half * 512, 512)],
                    start=(c == 0),
                    stop=(c == OC - 1),
                )
        u_rep = const.tile([128, H], f32)
        for half in range(2):
            nc.scalar.copy(u_rep[:, ds(half * 512, 512)], pb[half][:])

        # --- energies: fused multiply + free-axis reduce on DVE
        # out = (in0 * 1.0) * in1, accum_out = sum(out)
        e_loc = work.tile([128, TPT], f32)
        for k in range(NCHUNK):
            for s_ in range(SPC):
                t_idx = k * SPC + s_
                prod = scr.tile([128, H], f32, tag="prod")
                nc.vector.scalar_tensor_tensor(
                    out=prod[:], in0=q_sb[k][:, ds(s_ * H, H)], scalar=1.0,
                    in1=u_rep[:], op0=OP.mult, op1=OP.mult,
                    accum_out=e_loc[:, ds(t_idx, 1)],
                )

        # --- per-partition softmax pieces, packed with stats
        ot = work.tile([128, TPT + 2], f32)
        nc.vector.tensor_reduce(
            ot[:, ds(TPT, 1)], e_loc[:], axis=AX.X, op=OP.max, negate=True
        )
        nc.scalar.activation(
            ot[:, ds(0, TPT)], e_loc[:], mybir.ActivationFunctionType.Exp,
            bias=ot[:, ds(TPT, 1)], scale=1.0, accum_out=ot[:, ds(TPT + 1, 1)],
        )
        nc.sync.dma_start(outp[:], ot[:])

    nc.compile()
    return nc


def _get_nc():
    if "nc" not in _cached:
        _cached["nc"] = _build()
    return _cached["nc"]


def make_in_maps(question, W, v):
    qn = np.ascontiguousarray(np.asarray(question, dtype=np.float32))
    Wn = np.ascontiguousarray(np.asarray(W, dtype=np.float32))
    vn = np.ascontiguousarray(np.asarray(v, dtype=np.float32))
    # wcat[o, oc*H + j] = W[oc*128 + o, j] -> DMA is 128 x 32 KB contiguous
    wcat = np.ascontiguousarray(
        Wn.reshape(OC, 128, H).transpose(1, 0, 2).reshape(128, OC * H)
    )
    # vb[o, oc*128 + p] = v[128oc + o]: v chunks replicated across 128 cols
    vbm = np.ascontiguousarray(
        np.broadcast_to(
            vn.reshape(OC, 128).T[:, :, None], (128, OC, 128)
        ).reshape(128, OC * 128)
    )
    in_maps = []
    for r in range(NCORES):
        in_maps.append(
            {
                "q": qn[r * TPC : (r + 1) * TPC],  # contiguous row-slice view
                "wcat": wcat,
                "vb": vbm,
            }
        )
    return in_maps


def run(question, W, v, **spmd_kwargs):
    """Run the SPMD kernel; returns (out [S] fp32, BassKernelResults)."""
    from concourse.bass_utils import run_bass_kernel_spmd

    nc = _get_nc()
    in_maps = make_in_maps(question, W, v)
    res = run_bass_kernel_spmd(nc, in_maps, core_ids=list(range(NCORES)), **spmd_kwargs)
    blocks = np.stack(
        [
            np.asarray(res.results[r]["outp"], dtype=np.float64).reshape(
                128, TPT + 2
            )
            for r in range(NCORES)
        ]
    )  # [8, 128, 34]; token of (r, p, t) = r*4096 + 32p + t
    p_un = blocks[:, :, :TPT]
    m = -blocks[:, :, TPT]
    sums = blocks[:, :, TPT + 1]
    M = m.max()
    wgt = np.exp(m - M)
    Sg = (sums * wgt).sum()
    out = (p_un * (wgt / Sg)[:, :, None]).reshape(S)
    return out.astype(np.float32), res


def kernel(question, W, b, v):
    out, _ = run(question, W, v)
    return out.reshape(1, 1, S)
